# revision 1
# baseline (speedup 1.0000x reference)
"""Sliding-window GQA attention (B=1, S=4096, HID=1024, H=16, KV=4, D=64, W=512)
sharded across 8 trn2 NeuronCores by sequence (512 query rows/core + 512-row
k/v halo recomputed locally; core 0's halo is zero-padded and its softmax
denominator corrected through the sink term).

Self-contained: takes full inputs, shards on host, runs one SPMD Bass kernel
on cores 0-7, reassembles the full output.
"""
import sys
sys.path.insert(0, '/opt/trn_rl_repo')
import numpy as np

import concourse.bass as bass
import concourse.bacc as bacc
import concourse.hw_specs as _hw_specs

# Route every Ln/Exp activation to the single set that contains both
# ("natural_log_exp_and_others"), so the scheduler's interleaving of Ln and
# Exp ops never forces an ACT table reload (~2.7us each).
_orig_get_act_tables = _hw_specs.get_activation_tables


def _merged_act_tables(arch):
    t = dict(_orig_get_act_tables(arch))
    strip = {mybir.ActivationFunctionType.Ln, mybir.ActivationFunctionType.Exp,
             mybir.ActivationFunctionType.Square}
    for name, fns in t.items():
        if name != "natural_log_exp_and_others":
            t[name] = fns - strip
    return t


bacc.get_activation_tables = _merged_act_tables
import concourse.tile as tile
import concourse.mybir as mybir
from concourse.bass_utils import run_bass_kernel_spmd

F32 = mybir.dt.float32
F32R = mybir.dt.float32r
AF = mybir.ActivationFunctionType
OP = mybir.AluOpType

B, S, HID = 1, 4096, 1024
H, KV, D = 16, 4, 64
WINDOW = 512
EPS = 1e-5
NEG = -1e9
SCALE = 0.125  # 1/sqrt(D)
NCORE = 8
SLOC = 512    # query rows per core
SKV = 1024    # k/v rows per core (halo + own)

_cache = {}


def _build(phases="ABC"):
    nc = bacc.Bacc("TRN2", target_bir_lowering=False, debug=False, num_devices=NCORE)

    xT = nc.dram_tensor("xT", [HID, SKV], F32R, kind="ExternalInput").ap()
    wqT = nc.dram_tensor("wqT", [HID, HID], F32R, kind="ExternalInput").ap()
    wkvT = nc.dram_tensor("wkvT", [HID, 512], F32R, kind="ExternalInput").ap()
    woT = nc.dram_tensor("woT", [HID, HID], F32R, kind="ExternalInput").ap()
    cqd = nc.dram_tensor("cq", [SLOC, D], F32, kind="ExternalInput").ap()
    sqd = nc.dram_tensor("sq", [SLOC, D], F32, kind="ExternalInput").ap()
    ckd = nc.dram_tensor("ck", [SKV, D], F32, kind="ExternalInput").ap()
    skd = nc.dram_tensor("sk", [SKV, D], F32, kind="ExternalInput").ap()
    maskd = nc.dram_tensor("masks", [128, 4 * 256], F32R, kind="ExternalInput").ap()
    identd = nc.dram_tensor("ident", [128, 128], F32R, kind="ExternalInput").ap()
    onesd = nc.dram_tensor("onescol", [128, 1], F32R, kind="ExternalInput").ap()
    sinkd = nc.dram_tensor("sink_rhs", [16, 512], F32R, kind="ExternalInput").ap()
    vzrowd = nc.dram_tensor("vzrow", [1, 260], F32R, kind="ExternalInput").ap()
    vonesd = nc.dram_tensor("vones", [SKV, 1], F32R, kind="ExternalInput").ap()
    outd = nc.dram_tensor("out", [SLOC, HID], F32, kind="ExternalOutput").ap()

    with tile.TileContext(nc) as tc:
        with tc.tile_pool(name="const", bufs=1) as cpool, \
             tc.tile_pool(name="persist", bufs=1) as pers:

            # ---------------- tiny constants ----------------
            ident = cpool.tile([128, 128], F32R, tag="ident")
            nc.sync.dma_start(ident[:], identd)
            onescol = cpool.tile([128, 1], F32R, tag="ones")
            nc.sync.dma_start(onescol[:], onesd)
            epsc = cpool.tile([128, 1], F32, tag="epsc")
            nc.vector.memset(epsc[:], EPS)

            # persistent products of phase A
            v_sb = [pers.tile([128, 260], F32R, tag=f"v{st}", name=f"v{st}") for st in range(8)]
            vz_sb = {st: pers.tile([128, 260], F32R, tag=f"vz{st}", name=f"vz{st}") for st in (0, 2)}
            qT_sb = [pers.tile([128, 512], F32R, tag=f"qT{db}", name=f"qT{db}") for db in range(8)]
            kTd_sb = [pers.tile([128, SKV], F32R, tag=f"kT{kv}", name=f"kT{kv}") for kv in range(4)]
            aoT_sb = [pers.tile([128, 512], F32R, tag=f"aoT{p}", name=f"aoT{p}") for p in range(8)]

            with tc.tile_pool(name="xpool", bufs=1) as xpool:
                xT_sb = []
                for kb in range(8):
                    t = xpool.tile([128, SKV], F32R, tag=f"xT{kb}", name=f"xT{kb}")
                    eng = nc.sync if kb % 2 == 0 else nc.scalar
                    eng.dma_start(t[:], xT[kb * 128:(kb + 1) * 128, :])
                    xT_sb.append(t)

                # ======== phase A1: q projection + norm + rope + transpose ========
                with tc.tile_pool(name="wqp", bufs=1) as wqp, \
                     tc.tile_pool(name="tAq", bufs=3) as tA, \
                     tc.tile_pool(name="psmmq", bufs=4, space="PSUM") as psmm, \
                     tc.tile_pool(name="pstq", bufs=1, space="PSUM") as pst:
                    wq_sb = []
                    for kb in range(8):
                        t = wqp.tile([128, HID], F32R, tag=f"wq{kb}", name=f"wq{kb}")
                        eng = nc.sync if kb % 2 == 0 else nc.scalar
                        eng.dma_start(t[:], wqT[kb * 128:(kb + 1) * 128, :])
                        wq_sb.append(t)
                    cq_sb = []
                    sq_sb = []
                    for i in range(4):
                        t = wqp.tile([128, D], F32, tag=f"cq{i}", name=f"cq{i}")
                        nc.sync.dma_start(t[:], cqd[i * 128:(i + 1) * 128, :])
                        cq_sb.append(t)
                        t = wqp.tile([128, D], F32, tag=f"sq{i}", name=f"sq{i}")
                        nc.sync.dma_start(t[:], sqd[i * 128:(i + 1) * 128, :])
                        sq_sb.append(t)

                    def q_stage1(st, hf):
                        qp = psmm.tile([128, 512], F32, tag="qp", name=f"qp{st}{hf}")
                        for kb in range(8):
                            nc.tensor.matmul(qp[:], xT_sb[kb][:, st * 128:(st + 1) * 128],
                                             wq_sb[kb][:, hf * 512:(hf + 1) * 512],
                                             start=(kb == 0), stop=(kb == 7))
                        sqq = tA.tile([128, 512], F32, tag="sqq", name=f"sqq{st}{hf}")
                        nc.scalar.activation(sqq[:], qp[:], AF.Square)
                        msq = tA.tile([128, 8], F32, tag="msq", name=f"msq{st}{hf}")
                        nc.vector.tensor_reduce(
                            msq[:], sqq[:].rearrange("p (h d) -> p h d", h=8),
                            axis=mybir.AxisListType.X, op=OP.add)
                        return qp, msq

                    def q_stage2(st, hf, qp, msq, qTps):
                        lnq = tA.tile([128, 8], F32, tag="lnq", name=f"lnq{st}{hf}")
                        nc.scalar.activation(lnq[:], msq[:], AF.Ln, bias=epsc[:],
                                             scale=1.0 / D)
                        invq = tA.tile([128, 8], F32, tag="invq", name=f"invq{st}{hf}")
                        nc.scalar.activation(invq[:], lnq[:], AF.Exp, scale=-0.5)
                        qn = tA.tile([128, 512], F32, tag="qn", name=f"qn{st}{hf}")
                        qnv = qn[:].rearrange("p (h d) -> p h d", h=8)
                        nc.vector.tensor_mul(
                            qnv, qp[:].rearrange("p (h d) -> p h d", h=8),
                            invq[:].unsqueeze(2).broadcast_to([128, 8, D]))
                        q1 = tA.tile([128, 512], F32R, tag="q1", name=f"q1_{st}{hf}")
                        q1v = q1[:].rearrange("p (h d) -> p h d", h=8)
                        ct = cq_sb[st - 4]
                        stt = sq_sb[st - 4]
                        nc.vector.tensor_mul(
                            q1v, qnv, ct[:].unsqueeze(1).broadcast_to([128, 8, D]))
                        q2 = tA.tile([128, 512], F32R, tag="q2", name=f"q2_{st}{hf}")
                        nc.vector.tensor_mul(
                            q2[:].rearrange("p (h a j) -> p h a j", h=8, a=2),
                            qnv.rearrange("p h (a j) -> p h a j", a=2)[:, :, ::-1, :],
                            stt[:].rearrange("p (a j) -> p a j", a=2)
                            .unsqueeze(1).broadcast_to([128, 8, 2, 32]))
                        j = st - 4
                        for db4 in range(4):
                            sl = slice(db4 * 128, (db4 + 1) * 128)
                            nc.tensor.matmul(qTps[db4][:, j * 128:(j + 1) * 128],
                                             q1[:, sl], ident[:],
                                             is_transpose=True, start=True, stop=False)
                            nc.tensor.matmul(qTps[db4][:, j * 128:(j + 1) * 128],
                                             q2[:, sl], ident[:],
                                             is_transpose=True, start=False, stop=True)

                    for hf in range(2):
                        qTps = [pst.tile([128, 512], F32R, tag=f"qTps{d}",
                                         name=f"qTps{hf}_{d}") for d in range(4)]
                        pend = None
                        for st in range(4, 8):
                            cur = (st, hf, *q_stage1(st, hf))
                            if pend is not None:
                                q_stage2(*pend, qTps)
                            pend = cur
                        q_stage2(*pend, qTps)
                        for db4 in range(4):
                            nc.vector.tensor_copy(qT_sb[hf * 4 + db4][:], qTps[db4][:])

                # ======== phase A2: k/v projection + norm + rope + transpose ========
                with tc.tile_pool(name="wkp", bufs=1) as wkp, \
                     tc.tile_pool(name="tAk", bufs=3) as tA, \
                     tc.tile_pool(name="psmmk", bufs=4, space="PSUM") as psmm, \
                     tc.tile_pool(name="pstk", bufs=1, space="PSUM") as pst:
                    wkv_sb = []
                    for kb in range(8):
                        t = wkp.tile([128, 512], F32R, tag=f"wkv{kb}", name=f"wkv{kb}")
                        eng = nc.sync if kb % 2 == 0 else nc.scalar
                        eng.dma_start(t[:], wkvT[kb * 128:(kb + 1) * 128, :])
                        wkv_sb.append(t)
                    ck_sb = []
                    sk_sb = []
                    for i in range(8):
                        t = wkp.tile([128, D], F32, tag=f"ck{i}", name=f"ck{i}")
                        nc.sync.dma_start(t[:], ckd[i * 128:(i + 1) * 128, :])
                        ck_sb.append(t)
                        t = wkp.tile([128, D], F32, tag=f"sk{i}", name=f"sk{i}")
                        nc.sync.dma_start(t[:], skd[i * 128:(i + 1) * 128, :])
                        sk_sb.append(t)

                    def k_stage1(st):
                        kvp = psmm.tile([128, 512], F32, tag="kvp", name=f"kvp{st}")
                        for kb in range(8):
                            nc.tensor.matmul(kvp[:], xT_sb[kb][:, st * 128:(st + 1) * 128],
                                             wkv_sb[kb][:], start=(kb == 0), stop=(kb == 7))
                        nc.scalar.copy(
                            v_sb[st][:].rearrange("p (h d) -> p h d", d=65)[:, :, 0:64],
                            kvp[:, 256:512].rearrange("p (h d) -> p h d", d=64))
                        nc.sync.dma_start(
                            v_sb[st][:].rearrange("p (h d) -> p h d", d=65)[:, :, 64:65],
                            vonesd[st * 128:(st + 1) * 128, 0:1].unsqueeze(1)
                            .broadcast_to([128, KV, 1]))
                        if st in (0, 2):
                            nc.vector.tensor_copy(vz_sb[st][:], v_sb[st][:])
                            nc.sync.dma_start(vz_sb[st][0:1, :], vzrowd)
                        sqk = tA.tile([128, 256], F32, tag="sqk", name=f"sqk{st}")
                        nc.scalar.activation(sqk[:], kvp[:, 0:256], AF.Square)
                        msk = tA.tile([128, KV], F32, tag="msk", name=f"msk{st}")
                        nc.vector.tensor_reduce(
                            msk[:], sqk[:].rearrange("p (h d) -> p h d", h=KV),
                            axis=mybir.AxisListType.X, op=OP.add)
                        return kvp, msk

                    def k_stage2(st, kvp, msk, kTps):
                        lnk = tA.tile([128, KV], F32, tag="lnk", name=f"lnk{st}")
                        nc.scalar.activation(lnk[:], msk[:], AF.Ln, bias=epsc[:],
                                             scale=1.0 / D)
                        invk = tA.tile([128, KV], F32, tag="invk", name=f"invk{st}")
                        nc.scalar.activation(invk[:], lnk[:], AF.Exp, scale=-0.5)
                        kn = tA.tile([128, 256], F32, tag="kn", name=f"kn{st}")
                        knv = kn[:].rearrange("p (h d) -> p h d", h=KV)
                        nc.vector.tensor_mul(
                            knv, kvp[:, 0:256].rearrange("p (h d) -> p h d", h=KV),
                            invk[:].unsqueeze(2).broadcast_to([128, KV, D]))
                        k1 = tA.tile([128, 256], F32R, tag="k1", name=f"k1_{st}")
                        k1v = k1[:].rearrange("p (h d) -> p h d", h=KV)
                        nc.vector.tensor_mul(
                            k1v, knv, ck_sb[st][:].unsqueeze(1).broadcast_to([128, KV, D]))
                        k2 = tA.tile([128, 256], F32R, tag="k2", name=f"k2_{st}")
                        nc.vector.tensor_mul(
                            k2[:].rearrange("p (h a j) -> p h a j", h=KV, a=2),
                            knv.rearrange("p h (a j) -> p h a j", a=2)[:, :, ::-1, :],
                            sk_sb[st][:].rearrange("p (a j) -> p a j", a=2)
                            .unsqueeze(1).broadcast_to([128, KV, 2, 32]))
                        j = st % 4
                        for kv in range(4):
                            sl = slice(kv * 64, (kv + 1) * 64)
                            nc.tensor.matmul(kTps[kv][:, j * 128:(j + 1) * 128],
                                             k1[:, sl], ident[:],
                                             is_transpose=True, start=True, stop=False)
                            nc.tensor.matmul(kTps[kv][:, j * 128:(j + 1) * 128],
                                             k2[:, sl], ident[:],
                                             is_transpose=True, start=False, stop=True)

                    for sh in range(2):
                        kTps = [pst.tile([64, 512], F32R, tag=f"kTps{kv}",
                                         name=f"kTps{sh}_{kv}") for kv in range(4)]
                        pendk = None
                        for st4 in range(4):
                            st = sh * 4 + st4
                            curk = (st, *k_stage1(st))
                            if pendk is not None:
                                k_stage2(*pendk, kTps)
                            pendk = curk
                        k_stage2(*pendk, kTps)
                        for kv in range(4):
                            nc.vector.tensor_copy(
                                kTd_sb[kv][0:64, sh * 512:(sh + 1) * 512], kTps[kv][:])
                            nc.sync.dma_start(kTd_sb[kv][64:128, sh * 512:(sh + 1) * 512],
                                              kTd_sb[kv][0:64, sh * 512:(sh + 1) * 512])

            # ======== phase B: attention;  phase C: out-projection ========
            with tc.tile_pool(name="wB", bufs=1) as wB, \
                 tc.tile_pool(name="sbB", bufs=2) as sbB:
                masks = wB.tile([128, 4 * 256], F32R, tag="masks")
                nc.sync.dma_start(masks[:], maskd)
                sinkr = wB.tile([16, 512], F32R, tag="sinkr")
                nc.sync.dma_start(sinkr[:], sinkd)
                woT_sb = []
                for kb in range(8):
                    t = wB.tile([128, HID], F32R, tag=f"wo{kb}", name=f"wo{kb}")
                    eng = nc.sync if kb % 2 == 0 else nc.scalar
                    eng.dma_start(t[:], woT[kb * 128:(kb + 1) * 128, :])
                    woT_sb.append(t)

                MASKED = {0: 0, 1: 1, 4: 2, 5: 3}
                den_sb = wB.tile([16, 512], F32R, tag="den_sb")
                if "B" not in phases:
                    nc.vector.memset(den_sb[:].bitcast(F32), 1.0)
                    for p in range(8):
                        nc.vector.memset(aoT_sb[p][:].bitcast(F32), 0.0)
                rec_sb = wB.tile([16, 512], F32R, tag="rec_sb")
                with tc.tile_pool(name="psp", bufs=2, space="PSUM") as psp, \
                     tc.tile_pool(name="psav", bufs=2, space="PSUM") as psav:
                  for p in (range(8) if "B" in phases else []):
                      kv = p // 2
                      for Q in range(2):
                          psb = sbB.tile([128, 3072], F32R, tag="psb")
                          for h2 in range(2):
                              b = 64 * h2
                              pp = psp.tile([128, 1536], F32, tag="pp")
                              nc.tensor.matmul(pp[:, 0:512], ident[:], masks[:, 0:512],
                                               start=True, stop=False)
                              nc.tensor.matmul(pp[:, 1024:1536], ident[:], masks[:, 512:1024],
                                               start=True, stop=False)
                              for nu in range(6):
                                  kap = 2 * Q + nu
                                  lhsT = kTd_sb[kv][b:b + 64, kap * 128:(kap + 1) * 128]
                                  rhs = qT_sb[p][b:b + 64, Q * 256:(Q + 1) * 256]
                                  unm = nu in (2, 3)
                                  nc.tensor.matmul(pp[:, nu * 256:(nu + 1) * 256], lhsT, rhs,
                                                   start=unm, stop=unm or nu in (1, 5))
                              nc.scalar.activation(psb[:, h2 * 1536:(h2 + 1) * 1536], pp[:],
                                                   AF.Exp, scale=SCALE)
                              # sink row: strip nu=0 row 0 is always fully masked ->
                              # overwrite with exp(sink) (+ core-0 halo correction)
                              nc.sync.dma_start(
                                  psb[0:1, h2 * 1536: h2 * 1536 + 256],
                                  sinkr[2 * p + Q: 2 * p + Q + 1, h2 * 256:(h2 + 1) * 256])
                          avp = psav.tile([65, 512], F32, tag="avp")
                          for h2 in range(2):
                              for nu in range(6):
                                  stk = 2 * Q + nu
                                  rhs = psb[:, h2 * 1536 + nu * 256: h2 * 1536 + (nu + 1) * 256]
                                  vt = vz_sb[stk] if (nu == 0 and stk in (0, 2)) else v_sb[stk]
                                  nc.tensor.matmul(avp[:, h2 * 256:(h2 + 1) * 256],
                                                   vt[:, kv * 65:(kv + 1) * 65], rhs,
                                                   start=(nu == 0), stop=(nu == 5))
                          # drain psum, collect denominators, and move av halves
                          # into the transposed head layout (partition relocation
                          # must go through an SBUF-source DMA)
                          av2 = sbB.tile([65, 512], F32R, tag="av2")
                          nc.vector.tensor_copy(av2[:], avp[:])
                          nc.sync.dma_start(den_sb[2 * p + Q: 2 * p + Q + 1, :], av2[64:65, :])
                          nc.sync.dma_start(aoT_sb[p][0:64, Q * 256:(Q + 1) * 256],
                                            av2[0:64, 0:256])
                          nc.sync.dma_start(aoT_sb[p][64:128, Q * 256:(Q + 1) * 256],
                                            av2[0:64, 256:512])
                # batched reciprocal of all 32 denominators, then normalize in place
                if "B" not in phases:
                    den_sb = den_sb  # no-op
                nc.vector.reciprocal(rec_sb[:].bitcast(F32), den_sb[:].bitcast(F32))
                rec1 = wB.tile([1, 16 * 512], F32R, tag="rec1")
                nc.sync.dma_start(rec1[:], rec_sb[:])
                onesrow = wB.tile([1, 128], F32R, tag="onesrow")
                nc.vector.memset(onesrow[:].bitcast(F32), 1.0)
                with tc.tile_pool(name="psrep", bufs=2, space="PSUM") as psrep:
                  for p in range(8):
                    for Q in range(2):
                        base = (2 * p + Q) * 512
                        repA = psrep.tile([128, 256], F32, tag="repA")
                        repB = psrep.tile([128, 256], F32, tag="repB")
                        nc.tensor.matmul(repA[:], onesrow[:],
                                         rec1[0:1, base: base + 256])
                        nc.tensor.matmul(repB[:], onesrow[:],
                                         rec1[0:1, base + 256: base + 512])
                        nc.vector.tensor_mul(aoT_sb[p][0:64, Q * 256:(Q + 1) * 256],
                                             aoT_sb[p][0:64, Q * 256:(Q + 1) * 256],
                                             repA[0:64, :])
                        nc.vector.tensor_mul(aoT_sb[p][64:128, Q * 256:(Q + 1) * 256],
                                             aoT_sb[p][64:128, Q * 256:(Q + 1) * 256],
                                             repB[64:128, :])

                with tc.tile_pool(name="psC", bufs=2, space="PSUM") as psC:
                  for sblk in (range(4) if "C" in phases else []):
                    for nh in range(2):
                        op = psC.tile([128, 512], F32, tag="op")
                        for kb in range(8):
                            nc.tensor.matmul(op[:], aoT_sb[kb][:, sblk * 128:(sblk + 1) * 128],
                                             woT_sb[kb][:, nh * 512:(nh + 1) * 512],
                                             start=(kb == 0), stop=(kb == 7))
                        osb = sbB.tile([128, 512], F32, tag="osb")
                        nc.scalar.copy(osb[:], op[:])
                        nc.sync.dma_start(
                            outd[sblk * 128:(sblk + 1) * 128, nh * 512:(nh + 1) * 512],
                            osb[:])

    nc.compile()
    return nc


def _prep_inputs(x, cos, sin, wq, wk, wv, wo, q_norm_w, k_norm_w, sinks):
    """Build the 8 per-core input maps."""
    x = np.asarray(x, np.float32).reshape(S, HID)
    cos = np.asarray(cos, np.float32)
    sin = np.asarray(sin, np.float32)
    wq = np.asarray(wq, np.float32)
    wk = np.asarray(wk, np.float32)
    wv = np.asarray(wv, np.float32)
    wo = np.asarray(wo, np.float32)
    qw = np.asarray(q_norm_w, np.float32)
    kw = np.asarray(k_norm_w, np.float32)
    sinks = np.asarray(sinks, np.float32)

    wqT = np.ascontiguousarray(wq.T)                      # [HID, H*D]
    wkvT = np.ascontiguousarray(np.concatenate([wk, wv], 0).T)  # [HID, 512]
    woT = np.ascontiguousarray(wo.T)                      # [H*D, HID]
    ident = np.eye(128, dtype=np.float32)
    onescol = np.ones((128, 1), np.float32)

    # rope coefficient tables with norm weight folded in
    # q'' [d] = qn[d]*w[d]*cos[d] + rot(qn*w)[d]*sin[d]
    #   rot(qn*w)[d<32] = -qn[d+32]*w[d+32]; rot[d>=32] = qn[d-32]*w[d-32]
    sgn = np.concatenate([-np.ones(32, np.float32), np.ones(32, np.float32)])
    wrot_q = np.concatenate([qw[32:], qw[:32]])
    wrot_k = np.concatenate([kw[32:], kw[:32]])
    cw_q = cos * qw[None, :]
    sw_q = sin * (sgn * wrot_q)[None, :]
    cw_k = cos * kw[None, :]
    sw_k = sin * (sgn * wrot_k)[None, :]

    # additive masks for partial nu blocks (order nu=0,1,4,5)
    r = np.arange(128)[:, None]
    c = np.arange(256)[None, :]
    mstack = []
    for nu in (0, 1, 4, 5):
        ij = c - r + 512 - 128 * nu
        allowed = (ij >= 0) & (ij < WINDOW)
        mstack.append(np.where(allowed, 0.0, NEG).astype(np.float32))
    masks = np.concatenate(mstack, 1)                     # [128, 1024]

    xT = np.ascontiguousarray(x.T)                        # [HID, S]
    vzrow = np.zeros((1, 260), np.float32)
    vzrow[0, 64::65] = 1.0
    esink = np.exp(sinks.astype(np.float64)).astype(np.float32)

    in_maps = []
    for core in range(NCORE):
        start = SLOC * core
        lo = start - WINDOW
        xt_loc = np.zeros((HID, SKV), np.float32)
        srclo = max(0, lo)
        xt_loc[:, srclo - lo:] = xT[:, srclo:start + SLOC]
        idx_k = np.clip(np.arange(lo, start + SLOC), 0, S - 1)
        # sink rhs: exp(sink) minus count of mask-allowed halo (j<0) keys
        sink_rhs = np.zeros((16, 512), np.float32)
        for p in range(8):
            for Qb in range(2):
                for h2 in range(2):
                    sink_rhs[p * 2 + Qb, h2 * 256:(h2 + 1) * 256] = esink[2 * p + h2]
        vones = np.ones((SKV, 1), np.float32)
        if core == 0:
            vones[:WINDOW] = 0.0
        in_maps.append(dict(
            xT=xt_loc,
            wqT=wqT, wkvT=wkvT, woT=woT,
            cq=np.ascontiguousarray(cw_q[start:start + SLOC]),
            sq=np.ascontiguousarray(sw_q[start:start + SLOC]),
            ck=np.ascontiguousarray(cw_k[idx_k]),
            sk=np.ascontiguousarray(sw_k[idx_k]),
            masks=masks, ident=ident, onescol=onescol,
            sink_rhs=sink_rhs, vzrow=vzrow, vones=vones,
        ))
    return in_maps


def kernel(x, cos, sin, wq, wk, wv, wo, q_norm_w, k_norm_w, sinks, **kw):
    if "nc" not in _cache:
        _cache["nc"] = _build()
    nc = _cache["nc"]
    in_maps = _prep_inputs(x, cos, sin, wq, wk, wv, wo, q_norm_w, k_norm_w, sinks)
    res = run_bass_kernel_spmd(nc, in_maps, core_ids=list(range(NCORE)), **kw)
    out = np.empty((S, HID), np.float32)
    for core in range(NCORE):
        out[core * SLOC:(core + 1) * SLOC] = res.results[core]["out"]
    if kw:
        _cache["last_results"] = res
    return out.reshape(B, S, HID)



# revision 15
# speedup vs baseline: 1.4505x; 1.4505x over previous
"""Sliding-window GQA attention (B=1, S=4096, HID=1024, H=16, KV=4, D=64, W=512)
sharded across 8 trn2 NeuronCores by sequence (512 query rows/core + 512-row
k/v halo recomputed locally; core 0's halo is zero-padded and its softmax
denominator corrected through the sink term).

Self-contained: takes full inputs, shards on host, runs one SPMD Bass kernel
on cores 0-7, reassembles the full output.
"""
import sys
sys.path.insert(0, '/opt/trn_rl_repo')
import numpy as np
import ml_dtypes  # noqa: F401  (registers bfloat16 with numpy)
BF16NP = np.dtype('bfloat16')

import concourse.bass as bass
import concourse.bacc as bacc
import concourse.hw_specs as _hw_specs

# Route every Ln/Exp activation to the single set that contains both
# ("natural_log_exp_and_others"), so the scheduler's interleaving of Ln and
# Exp ops never forces an ACT table reload (~2.7us each).
_orig_get_act_tables = _hw_specs.get_activation_tables


def _merged_act_tables(arch):
    t = dict(_orig_get_act_tables(arch))
    strip = {mybir.ActivationFunctionType.Ln, mybir.ActivationFunctionType.Exp,
             mybir.ActivationFunctionType.Square}
    for name, fns in t.items():
        if name != "natural_log_exp_and_others":
            t[name] = fns - strip
    return t


bacc.get_activation_tables = _merged_act_tables
import concourse.tile as tile
import concourse.mybir as mybir
from concourse.bass_utils import run_bass_kernel_spmd

F32 = mybir.dt.float32
F32R = mybir.dt.float32r
BF16 = mybir.dt.bfloat16
AF = mybir.ActivationFunctionType
OP = mybir.AluOpType

B, S, HID = 1, 4096, 1024
H, KV, D = 16, 4, 64
WINDOW = 512
EPS = 1e-5
NEG = -1e9
SCALE = 0.125  # 1/sqrt(D)
NCORE = 8
SLOC = 512    # query rows per core
SKV = 1024    # k/v rows per core (halo + own)

_cache = {}


def _build(phases="ABC"):
    nc = bacc.Bacc("TRN2", target_bir_lowering=False, debug=False, num_devices=NCORE)

    xT = nc.dram_tensor("xT", [HID, SKV], BF16, kind="ExternalInput").ap()
    wqT = nc.dram_tensor("wqT", [HID, HID], BF16, kind="ExternalInput").ap()
    wkvT = nc.dram_tensor("wkvT", [HID, 512], BF16, kind="ExternalInput").ap()
    woT = nc.dram_tensor("woT", [HID, HID], BF16, kind="ExternalInput").ap()
    cqsqd = nc.dram_tensor("cqsq", [128, 512], F32, kind="ExternalInput").ap()
    ckskd = nc.dram_tensor("cksk", [128, 1024], F32, kind="ExternalInput").ap()
    maskd = nc.dram_tensor("masks", [128, 4 * 256], BF16, kind="ExternalInput").ap()
    identd = nc.dram_tensor("ident", [128, 128], BF16, kind="ExternalInput").ap()
    identrd = nc.dram_tensor("identr", [128, 128], F32R, kind="ExternalInput").ap()
    sinkd = nc.dram_tensor("sink_rhs", [1, 16 * 512], F32R, kind="ExternalInput").ap()
    vonesd = nc.dram_tensor("vones", [SKV, 1], BF16, kind="ExternalInput").ap()
    outd = nc.dram_tensor("out", [SLOC, HID], F32, kind="ExternalOutput").ap()

    with tile.TileContext(nc) as tc:
        with tc.tile_pool(name="const", bufs=1) as cpool, \
             tc.tile_pool(name="persist", bufs=1) as pers:

            # ---------------- tiny constants (first: cheap, needed early) ----
            ident = cpool.tile([128, 128], BF16, tag="ident")
            nc.gpsimd.dma_start(ident[:], identd)
            identr = cpool.tile([128, 128], F32R, tag="identr")
            nc.gpsimd.dma_start(identr[:], identrd)
            epsc = cpool.tile([128, 1], F32, tag="epsc")
            nc.vector.memset(epsc[:], EPS)
            delta65 = cpool.tile([1, 65], F32R, tag="delta65")
            nc.vector.memset(delta65[:].bitcast(F32), 0.0)
            nc.vector.memset(delta65[0:1, 64:65].bitcast(F32), 1.0)

            # persistent products of phase A
            v_sb = [pers.tile([128, 260], BF16, tag=f"v{st}", name=f"v{st}") for st in range(8)]
            qT_sb = [pers.tile([128, 512], F32R, tag=f"qT{db}", name=f"qT{db}") for db in range(8)]
            kTd_sb = [pers.tile([128, SKV], F32R, tag=f"kT{kv}", name=f"kT{kv}") for kv in range(4)]
            aoT_sb = [pers.tile([128, 512], BF16, tag=f"aoT{p}", name=f"aoT{p}") for p in range(8)]

            with tc.tile_pool(name="xpool", bufs=1) as xpool, \
                 tc.tile_pool(name="wkp", bufs=1) as wkp, \
                 tc.tile_pool(name="wqp", bufs=1) as wqp:
                # ---- input DMAs, ordered by first use: x/wkv interleaved,
                # then rope-k, then rest of x, then cq/sq + wq, masks last.
                xT_sb = [None] * 8
                wkv_sb = [None] * 8

                def load_x(kb):
                    t = xpool.tile([128, SKV], BF16, tag=f"xT{kb}", name=f"xT{kb}")
                    eng = nc.sync if kb % 2 == 0 else nc.scalar
                    eng.dma_start(t[:], xT[kb * 128:(kb + 1) * 128, :])
                    xT_sb[kb] = t

                def load_wkv(kb):
                    t = wkp.tile([128, 512], BF16, tag=f"wkv{kb}", name=f"wkv{kb}")
                    eng = nc.scalar if kb % 2 == 0 else nc.sync
                    eng.dma_start(t[:], wkvT[kb * 128:(kb + 1) * 128, :])
                    wkv_sb[kb] = t

                for kb in range(4):
                    load_x(kb)
                    load_wkv(kb)
                cksk_sb = wkp.tile([128, 1024], F32, tag="cksk")
                nc.sync.dma_start(cksk_sb[:], ckskd)
                for kb in range(4, 8):
                    load_x(kb)
                    load_wkv(kb)
                cqsq_sb = wqp.tile([128, 512], F32, tag="cqsq")
                nc.sync.dma_start(cqsq_sb[:], cqsqd)
                wq_sb = []
                for kb in range(8):
                    t = wqp.tile([128, HID], BF16, tag=f"wq{kb}", name=f"wq{kb}")
                    nc.gpsimd.dma_start(t[:], wqT[kb * 128:(kb + 1) * 128, :])
                    wq_sb.append(t)

                # ======== phase A2: k/v projection + norm + rope + transpose ====
                with tc.tile_pool(name="tAk", bufs=3) as tA, \
                     tc.tile_pool(name="psmmk", bufs=4, space="PSUM") as psmm, \
                     tc.tile_pool(name="pstk", bufs=1, space="PSUM") as pst:

                    def k_stage2(st, kvp, msk, kTps):
                        lnk = tA.tile([128, KV], F32, tag="lnk", name=f"lnk{st}")
                        nc.scalar.activation(lnk[:], msk[:], AF.Ln, bias=epsc[:],
                                             scale=1.0 / D)
                        invk = tA.tile([128, KV], F32, tag="invk", name=f"invk{st}")
                        nc.scalar.activation(invk[:], lnk[:], AF.Exp, scale=-0.5)
                        kn = tA.tile([128, 256], F32, tag="kn", name=f"kn{st}")
                        knv = kn[:].rearrange("p (h d) -> p h d", h=KV)
                        nc.vector.tensor_mul(
                            knv, kvp[:, 0:256].rearrange("p (h d) -> p h d", h=KV),
                            invk[:].unsqueeze(2).broadcast_to([128, KV, D]))
                        k1 = tA.tile([128, 256], F32R, tag="k1", name=f"k1_{st}")
                        k1v = k1[:].rearrange("p (h d) -> p h d", h=KV)
                        ck = cksk_sb[:, st * 64:(st + 1) * 64]
                        sk = cksk_sb[:, 512 + st * 64: 512 + (st + 1) * 64]
                        nc.vector.tensor_mul(
                            k1v, knv, ck.unsqueeze(1).broadcast_to([128, KV, D]))
                        k2 = tA.tile([128, 256], F32R, tag="k2", name=f"k2_{st}")
                        nc.vector.tensor_mul(
                            k2[:].rearrange("p (h a j) -> p h a j", h=KV, a=2),
                            knv.rearrange("p h (a j) -> p h a j", a=2)[:, :, ::-1, :],
                            sk.rearrange("p (a j) -> p a j", a=2)
                            .unsqueeze(1).broadcast_to([128, KV, 2, 32]))
                        kr = tA.tile([128, 256], F32R, tag="kr", name=f"kr{st}")
                        nc.vector.tensor_add(kr[:], k1[:], k2[:])
                        j = st % 4
                        for kv in range(4):
                            sl = slice(kv * 64, (kv + 1) * 64)
                            nc.tensor.matmul(kTps[kv][:, j * 128:(j + 1) * 128],
                                             kr[:, sl], identr[:],
                                             is_transpose=True, start=True, stop=True)

                    def k_stage1(st):
                        kvp = psmm.tile([128, 512], F32, tag="kvp", name=f"kvp{st}")
                        for kb in range(8):
                            nc.tensor.matmul(kvp[:], xT_sb[kb][:, st * 128:(st + 1) * 128],
                                             wkv_sb[kb][:], start=(kb == 0), stop=(kb == 7))
                        nc.scalar.copy(
                            v_sb[st][:].rearrange("p (h d) -> p h d", d=65)[:, :, 0:64],
                            kvp[:, 256:512].rearrange("p (h d) -> p h d", d=64))
                        nc.gpsimd.dma_start(
                            v_sb[st][:].rearrange("p (h d) -> p h d", d=65)[:, :, 64:65],
                            vonesd[st * 128:(st + 1) * 128, 0:1].unsqueeze(1)
                            .broadcast_to([128, KV, 1]))
                        sqk = tA.tile([128, 256], F32, tag="sqk", name=f"sqk{st}")
                        nc.scalar.activation(sqk[:], kvp[:, 0:256], AF.Square)
                        msk = tA.tile([128, KV], F32, tag="msk", name=f"msk{st}")
                        nc.vector.tensor_reduce(
                            msk[:], sqk[:].rearrange("p (h d) -> p h d", h=KV),
                            axis=mybir.AxisListType.X, op=OP.add)
                        return kvp, msk

                    for sh in range(2):
                        kTps = [pst.tile([64, 512], F32R, tag=f"kTps{kv}",
                                         name=f"kTps{sh}_{kv}") for kv in range(4)]
                        pend = []
                        for st4 in range(4):
                            st = sh * 4 + st4
                            pend.append((st, *k_stage1(st)))
                            if len(pend) > 2:
                                k_stage2(*pend.pop(0), kTps)
                        for it in pend:
                            k_stage2(*it, kTps)
                        for kv in range(4):
                            nc.vector.tensor_copy(
                                kTd_sb[kv][0:64, sh * 512:(sh + 1) * 512], kTps[kv][:])
                            nc.gpsimd.dma_start(kTd_sb[kv][64:128, sh * 512:(sh + 1) * 512],
                                                kTd_sb[kv][0:64, sh * 512:(sh + 1) * 512])

                # ======== phase A1: q projection + norm + rope + transpose ======
                with tc.tile_pool(name="tAq", bufs=3) as tA, \
                     tc.tile_pool(name="psmmq", bufs=4, space="PSUM") as psmm, \
                     tc.tile_pool(name="pstq", bufs=1, space="PSUM") as pst:

                    def q_stage2(st, hf, qp, msq, qTps):
                        lnq = tA.tile([128, 8], F32, tag="lnq", name=f"lnq{st}{hf}")
                        nc.scalar.activation(lnq[:], msq[:], AF.Ln, bias=epsc[:],
                                             scale=1.0 / D)
                        invq = tA.tile([128, 8], F32, tag="invq", name=f"invq{st}{hf}")
                        nc.scalar.activation(invq[:], lnq[:], AF.Exp, scale=-0.5)
                        qn = tA.tile([128, 512], F32, tag="qn", name=f"qn{st}{hf}")
                        qnv = qn[:].rearrange("p (h d) -> p h d", h=8)
                        nc.vector.tensor_mul(
                            qnv, qp[:].rearrange("p (h d) -> p h d", h=8),
                            invq[:].unsqueeze(2).broadcast_to([128, 8, D]))
                        q1 = tA.tile([128, 512], F32R, tag="q1", name=f"q1_{st}{hf}")
                        q1v = q1[:].rearrange("p (h d) -> p h d", h=8)
                        i = st - 4
                        ct = cqsq_sb[:, i * 64:(i + 1) * 64]
                        stt = cqsq_sb[:, 256 + i * 64: 256 + (i + 1) * 64]
                        nc.vector.tensor_mul(
                            q1v, qnv, ct.unsqueeze(1).broadcast_to([128, 8, D]))
                        q2 = tA.tile([128, 512], F32R, tag="q2", name=f"q2_{st}{hf}")
                        nc.vector.tensor_mul(
                            q2[:].rearrange("p (h a j) -> p h a j", h=8, a=2),
                            qnv.rearrange("p h (a j) -> p h a j", a=2)[:, :, ::-1, :],
                            stt.rearrange("p (a j) -> p a j", a=2)
                            .unsqueeze(1).broadcast_to([128, 8, 2, 32]))
                        qr = tA.tile([128, 512], F32R, tag="qr", name=f"qr{st}{hf}")
                        nc.vector.tensor_add(qr[:], q1[:], q2[:])
                        j = st - 4
                        for db4 in range(4):
                            sl = slice(db4 * 128, (db4 + 1) * 128)
                            nc.tensor.matmul(qTps[db4][:, j * 128:(j + 1) * 128],
                                             qr[:, sl], identr[:],
                                             is_transpose=True, start=True, stop=True)

                    def q_stage1(st, hf):
                        qp = psmm.tile([128, 512], F32, tag="qp", name=f"qp{st}{hf}")
                        for kb in range(8):
                            nc.tensor.matmul(qp[:], xT_sb[kb][:, st * 128:(st + 1) * 128],
                                             wq_sb[kb][:, hf * 512:(hf + 1) * 512],
                                             start=(kb == 0), stop=(kb == 7))
                        sqq = tA.tile([128, 512], F32, tag="sqq", name=f"sqq{st}{hf}")
                        nc.scalar.activation(sqq[:], qp[:], AF.Square)
                        msq = tA.tile([128, 8], F32, tag="msq", name=f"msq{st}{hf}")
                        nc.vector.tensor_reduce(
                            msq[:], sqq[:].rearrange("p (h d) -> p h d", h=8),
                            axis=mybir.AxisListType.X, op=OP.add)
                        return qp, msq

                    for hf in range(2):
                        qTps = [pst.tile([128, 512], F32R, tag=f"qTps{d}",
                                         name=f"qTps{hf}_{d}") for d in range(4)]
                        pend = []
                        for st in range(4, 8):
                            pend.append((st, hf, *q_stage1(st, hf)))
                            if len(pend) > 2:
                                q_stage2(*pend.pop(0), qTps)
                        for it in pend:
                            q_stage2(*it, qTps)
                        for db4 in range(4):
                            nc.vector.tensor_copy(qT_sb[hf * 4 + db4][:], qTps[db4][:])

            # ======== phase B: attention;  phase C: out-projection ========
            with tc.tile_pool(name="wB", bufs=1) as wB, \
                 tc.tile_pool(name="sbB", bufs=2) as sbB:
                masks = wB.tile([128, 4 * 256], BF16, tag="masks")
                nc.gpsimd.dma_start(masks[:], maskd)
                sinkr = wB.tile([1, 16 * 512], F32R, tag="sinkr")
                nc.gpsimd.dma_start(sinkr[:], sinkd)
                woT_sb = []
                for kb in range(8):
                    t = wB.tile([128, HID], BF16, tag=f"wo{kb}", name=f"wo{kb}")
                    nc.gpsimd.dma_start(t[:], woT[kb * 128:(kb + 1) * 128, :])
                    woT_sb.append(t)

                den_sb = wB.tile([16, 512], F32R, tag="den_sb")
                if "B" not in phases:
                    nc.vector.memset(den_sb[:].bitcast(F32), 1.0)
                    for p in range(8):
                        nc.vector.memset(aoT_sb[p][:].bitcast(F32), 0.0)
                rec_sb = wB.tile([16, 512], F32R, tag="rec_sb")
                with tc.tile_pool(name="psp", bufs=2, space="PSUM") as psp, \
                     tc.tile_pool(name="psav", bufs=2, space="PSUM") as psav:
                  for p in (range(8) if "B" in phases else []):
                      kv = p // 2
                      for Q in range(2):
                          psb = sbB.tile([128, 3072], BF16, tag="psb")
                          for h2 in range(2):
                              b = 64 * h2
                              ppA = psp.tile([128, 1024], F32, tag="ppA")
                              ppB = psp.tile([128, 512], F32, tag="ppB")
                              nc.tensor.matmul(ppA[:, 0:512], ident[:], masks[:, 0:512],
                                               start=True, stop=False)
                              nc.tensor.matmul(ppB[:], ident[:], masks[:, 512:1024],
                                               start=True, stop=False)
                              for nu in range(6):
                                  kap = 2 * Q + nu
                                  lhsT = kTd_sb[kv][b:b + 64, kap * 128:(kap + 1) * 128]
                                  rhs = qT_sb[p][b:b + 64, Q * 256:(Q + 1) * 256]
                                  unm = nu in (2, 3)
                                  tgt = (ppA[:, nu * 256:(nu + 1) * 256] if nu < 4
                                         else ppB[:, (nu - 4) * 256:(nu - 3) * 256])
                                  nc.tensor.matmul(tgt, lhsT, rhs,
                                                   start=unm, stop=unm or nu in (1, 5))
                              nc.scalar.activation(psb[:, h2 * 1536: h2 * 1536 + 1024],
                                                   ppA[:], AF.Exp, scale=SCALE)
                              nc.scalar.activation(psb[:, h2 * 1536 + 1024: h2 * 1536 + 1536],
                                                   ppB[:], AF.Exp, scale=SCALE)
                          avp = psav.tile([65, 512], F32, tag="avp")
                          for h2 in range(2):
                              for nu in range(6):
                                  stk = 2 * Q + nu
                                  rhs = psb[:, h2 * 1536 + nu * 256: h2 * 1536 + (nu + 1) * 256]
                                  nc.tensor.matmul(avp[:, h2 * 256:(h2 + 1) * 256],
                                                   v_sb[stk][:, kv * 65:(kv + 1) * 65], rhs,
                                                   start=(nu == 0), stop=False)
                              # sink + core-0 halo correction folded into the
                              # denominator row (row 64) via a rank-1 matmul
                              nc.tensor.matmul(avp[:, h2 * 256:(h2 + 1) * 256],
                                               delta65[:],
                                               sinkr[0:1, (2 * p + Q) * 512 + h2 * 256:
                                                     (2 * p + Q) * 512 + (h2 + 1) * 256],
                                               start=False, stop=True)
                          # drain psum, collect denominators, and move av halves
                          # into the transposed head layout (partition relocation
                          # must go through an SBUF-source DMA)
                          av2 = sbB.tile([64, 512], BF16, tag="av2")
                          nc.vector.tensor_copy(av2[:], avp[0:64, :])
                          denrow = sbB.tile([65, 512], F32R, tag="denrow")
                          nc.scalar.copy(denrow[64:65, :].bitcast(F32), avp[64:65, :])
                          nc.sync.dma_start(den_sb[2 * p + Q: 2 * p + Q + 1, :],
                                            denrow[64:65, :])
                          nc.sync.dma_start(aoT_sb[p][0:64, Q * 256:(Q + 1) * 256],
                                            av2[:, 0:256])
                          nc.sync.dma_start(aoT_sb[p][64:128, Q * 256:(Q + 1) * 256],
                                            av2[:, 256:512])
                # batched reciprocal of all 32 denominators, then normalize in place
                nc.vector.reciprocal(rec_sb[:].bitcast(F32), den_sb[:].bitcast(F32))
                rec1 = wB.tile([1, 16 * 512], F32R, tag="rec1")
                nc.sync.dma_start(rec1[:], rec_sb[:])
                onesrow = wB.tile([1, 128], F32R, tag="onesrow")
                nc.vector.memset(onesrow[:].bitcast(F32), 1.0)
                with tc.tile_pool(name="psrep", bufs=2, space="PSUM") as psrep:
                  for p in range(8):
                    for Q in range(2):
                        base = (2 * p + Q) * 512
                        repA = psrep.tile([128, 256], F32, tag="repA")
                        repB = psrep.tile([128, 256], F32, tag="repB")
                        nc.tensor.matmul(repA[:], onesrow[:],
                                         rec1[0:1, base: base + 256])
                        nc.tensor.matmul(repB[:], onesrow[:],
                                         rec1[0:1, base + 256: base + 512])
                        nc.vector.tensor_mul(aoT_sb[p][0:64, Q * 256:(Q + 1) * 256],
                                             aoT_sb[p][0:64, Q * 256:(Q + 1) * 256],
                                             repA[0:64, :])
                        nc.vector.tensor_mul(aoT_sb[p][64:128, Q * 256:(Q + 1) * 256],
                                             aoT_sb[p][64:128, Q * 256:(Q + 1) * 256],
                                             repB[64:128, :])

                with tc.tile_pool(name="psC", bufs=2, space="PSUM") as psC:
                  for sblk in (range(4) if "C" in phases else []):
                    for nh in range(2):
                        op = psC.tile([128, 512], F32, tag="op")
                        for kb in range(8):
                            nc.tensor.matmul(op[:], aoT_sb[kb][:, sblk * 128:(sblk + 1) * 128],
                                             woT_sb[kb][:, nh * 512:(nh + 1) * 512],
                                             start=(kb == 0), stop=(kb == 7))
                        osb = sbB.tile([128, 512], F32, tag="osb")
                        nc.scalar.copy(osb[:], op[:])
                        eng = nc.sync if (sblk + nh) % 2 == 0 else nc.scalar
                        eng.dma_start(
                            outd[sblk * 128:(sblk + 1) * 128, nh * 512:(nh + 1) * 512],
                            osb[:])

    nc.compile()
    return nc


def _prep_inputs(x, cos, sin, wq, wk, wv, wo, q_norm_w, k_norm_w, sinks):
    """Build the 8 per-core input maps."""
    x = np.asarray(x, np.float32).reshape(S, HID)
    cos = np.asarray(cos, np.float32)
    sin = np.asarray(sin, np.float32)
    wq = np.asarray(wq, np.float32)
    wk = np.asarray(wk, np.float32)
    wv = np.asarray(wv, np.float32)
    wo = np.asarray(wo, np.float32)
    qw = np.asarray(q_norm_w, np.float32)
    kw = np.asarray(k_norm_w, np.float32)
    sinks = np.asarray(sinks, np.float32)

    wqT = np.ascontiguousarray(wq.T)                      # [HID, H*D]
    wkvT = np.ascontiguousarray(np.concatenate([wk, wv], 0).T)  # [HID, 512]
    woT = np.ascontiguousarray(wo.T)                      # [H*D, HID]
    ident = np.eye(128, dtype=np.float32)

    # rope coefficient tables with norm weight folded in
    # q'' [d] = qn[d]*w[d]*cos[d] + rot(qn*w)[d]*sin[d]
    #   rot(qn*w)[d<32] = -qn[d+32]*w[d+32]; rot[d>=32] = qn[d-32]*w[d-32]
    sgn = np.concatenate([-np.ones(32, np.float32), np.ones(32, np.float32)])
    wrot_q = np.concatenate([qw[32:], qw[:32]])
    wrot_k = np.concatenate([kw[32:], kw[:32]])
    cw_q = cos * qw[None, :]
    sw_q = sin * (sgn * wrot_q)[None, :]
    cw_k = cos * kw[None, :]
    sw_k = sin * (sgn * wrot_k)[None, :]

    # additive masks for partial nu blocks (order nu=0,1,4,5)
    r = np.arange(128)[:, None]
    c = np.arange(256)[None, :]
    mstack = []
    for nu in (0, 1, 4, 5):
        ij = c - r + 512 - 128 * nu
        allowed = (ij >= 0) & (ij < WINDOW)
        mstack.append(np.where(allowed, 0.0, NEG).astype(np.float32))
    masks = np.concatenate(mstack, 1)                     # [128, 1024]

    xT = np.ascontiguousarray(x.T)                        # [HID, S]
    esink = np.exp(sinks.astype(np.float64)).astype(np.float32)

    in_maps = []
    for core in range(NCORE):
        start = SLOC * core
        lo = start - WINDOW
        xt_loc = np.zeros((HID, SKV), np.float32)
        srclo = max(0, lo)
        xt_loc[:, srclo - lo:] = xT[:, srclo:start + SLOC]
        idx_k = np.clip(np.arange(lo, start + SLOC), 0, S - 1)
        # sink rhs row: col (2p+Q)*512 + h2*256 + qq -> exp(sink[2p+h2]).
        # core 0: halo keys' denominator contributions are suppressed by
        # zeroing their v ones-column (vones), as large partial sums would
        # hit the reduced-precision psum accumulate.
        sink_rhs = np.zeros((1, 8192), np.float32)
        for p in range(8):
            for Qb in range(2):
                for h2 in range(2):
                    base = (2 * p + Qb) * 512 + h2 * 256
                    sink_rhs[0, base:base + 256] = esink[2 * p + h2]
        vones = np.ones((SKV, 1), np.float32)
        if core == 0:
            vones[:WINDOW] = 0.0
        # packed rope tables: [128, st-blocks*64] (cos blocks then sin blocks)
        cq_loc = cw_q[start:start + SLOC].reshape(4, 128, 64)
        sq_loc = sw_q[start:start + SLOC].reshape(4, 128, 64)
        cqsq = np.concatenate(
            [cq_loc[i] for i in range(4)] + [sq_loc[i] for i in range(4)], axis=1)
        ck_loc = cw_k[idx_k].reshape(8, 128, 64)
        sk_loc = sw_k[idx_k].reshape(8, 128, 64)
        cksk = np.concatenate(
            [ck_loc[i] for i in range(8)] + [sk_loc[i] for i in range(8)], axis=1)
        in_maps.append(dict(
            xT=xt_loc.astype(BF16NP),
            wqT=wqT.astype(BF16NP), wkvT=wkvT.astype(BF16NP),
            woT=woT.astype(BF16NP),
            cqsq=np.ascontiguousarray(cqsq),
            cksk=np.ascontiguousarray(cksk),
            masks=masks.astype(BF16NP), ident=ident.astype(BF16NP),
            identr=ident,
            sink_rhs=sink_rhs, vones=vones.astype(BF16NP),
        ))
    return in_maps


def kernel(x, cos, sin, wq, wk, wv, wo, q_norm_w, k_norm_w, sinks, **kw):
    if "nc" not in _cache:
        _cache["nc"] = _build()
    nc = _cache["nc"]
    in_maps = _prep_inputs(x, cos, sin, wq, wk, wv, wo, q_norm_w, k_norm_w, sinks)
    res = run_bass_kernel_spmd(nc, in_maps, core_ids=list(range(NCORE)), **kw)
    out = np.empty((S, HID), np.float32)
    for core in range(NCORE):
        out[core * SLOC:(core + 1) * SLOC] = res.results[core]["out"]
    if kw:
        _cache["last_results"] = res
    return out.reshape(B, S, HID)


# revision 16
# speedup vs baseline: 1.6013x; 1.1039x over previous
"""Sliding-window GQA attention (B=1, S=4096, HID=1024, H=16, KV=4, D=64, W=512)
sharded across 8 trn2 NeuronCores by sequence (512 query rows/core + 512-row
k/v halo recomputed locally; core 0's halo is zero-padded and its softmax
denominator corrected through the sink term).

Self-contained: takes full inputs, shards on host, runs one SPMD Bass kernel
on cores 0-7, reassembles the full output.
"""
import sys
sys.path.insert(0, '/opt/trn_rl_repo')
import numpy as np
import ml_dtypes  # noqa: F401  (registers bfloat16 with numpy)
BF16NP = np.dtype('bfloat16')

import concourse.bass as bass
import concourse.bacc as bacc
import concourse.hw_specs as _hw_specs

# Route every Ln/Exp activation to the single set that contains both
# ("natural_log_exp_and_others"), so the scheduler's interleaving of Ln and
# Exp ops never forces an ACT table reload (~2.7us each).
_orig_get_act_tables = _hw_specs.get_activation_tables


def _merged_act_tables(arch):
    t = dict(_orig_get_act_tables(arch))
    strip = {mybir.ActivationFunctionType.Ln, mybir.ActivationFunctionType.Exp,
             mybir.ActivationFunctionType.Square}
    for name, fns in t.items():
        if name != "natural_log_exp_and_others":
            t[name] = fns - strip
    return t


bacc.get_activation_tables = _merged_act_tables
import concourse.tile as tile
import concourse.mybir as mybir
from concourse.bass_utils import run_bass_kernel_spmd

F32 = mybir.dt.float32
F32R = mybir.dt.float32r
BF16 = mybir.dt.bfloat16
AF = mybir.ActivationFunctionType
OP = mybir.AluOpType

B, S, HID = 1, 4096, 1024
H, KV, D = 16, 4, 64
WINDOW = 512
EPS = 1e-5
NEG = -1e9
SCALE = 0.125  # 1/sqrt(D)
NCORE = 8
SLOC = 512    # query rows per core
SKV = 1024    # k/v rows per core (halo + own)

_cache = {}


def _build(phases="ABC"):
    nc = bacc.Bacc("TRN2", target_bir_lowering=False, debug=False, num_devices=NCORE)

    xT = nc.dram_tensor("xT", [HID, SKV], BF16, kind="ExternalInput").ap()
    wqT = nc.dram_tensor("wqT", [HID, HID], BF16, kind="ExternalInput").ap()
    wkvT = nc.dram_tensor("wkvT", [HID, 512], BF16, kind="ExternalInput").ap()
    woT = nc.dram_tensor("woT", [HID, HID], BF16, kind="ExternalInput").ap()
    cqsqd = nc.dram_tensor("cqsq", [128, 512], F32, kind="ExternalInput").ap()
    ckskd = nc.dram_tensor("cksk", [128, 1024], F32, kind="ExternalInput").ap()
    maskd = nc.dram_tensor("masks", [128, 4 * 256], BF16, kind="ExternalInput").ap()
    identd = nc.dram_tensor("ident", [128, 128], BF16, kind="ExternalInput").ap()
    identrd = nc.dram_tensor("identr", [128, 128], F32R, kind="ExternalInput").ap()
    sinkd = nc.dram_tensor("sink_rhs", [1, 16 * 512], F32R, kind="ExternalInput").ap()
    vonesd = nc.dram_tensor("vones", [SKV, 1], BF16, kind="ExternalInput").ap()
    outd = nc.dram_tensor("out", [SLOC, HID], F32, kind="ExternalOutput").ap()

    with tile.TileContext(nc) as tc:
        with tc.tile_pool(name="const", bufs=1) as cpool, \
             tc.tile_pool(name="persist", bufs=1) as pers:

            # ---------------- tiny constants (first: cheap, needed early) ----
            ident = cpool.tile([128, 128], BF16, tag="ident")
            nc.gpsimd.dma_start(ident[:], identd)
            identr = cpool.tile([128, 128], F32R, tag="identr")
            nc.gpsimd.dma_start(identr[:], identrd)
            epsc = cpool.tile([128, 1], F32, tag="epsc")
            nc.vector.memset(epsc[:], EPS)
            delta65 = cpool.tile([1, 65], F32R, tag="delta65")
            nc.vector.memset(delta65[:].bitcast(F32), 0.0)
            nc.vector.memset(delta65[0:1, 64:65].bitcast(F32), 1.0)

            # persistent products of phase A
            v_sb = [pers.tile([128, 260], BF16, tag=f"v{st}", name=f"v{st}") for st in range(8)]
            qT_sb = [pers.tile([128, 512], F32R, tag=f"qT{db}", name=f"qT{db}") for db in range(8)]
            kTd_sb = [pers.tile([128, SKV], F32R, tag=f"kT{kv}", name=f"kT{kv}") for kv in range(4)]
            aoT_sb = [pers.tile([128, 512], BF16, tag=f"aoT{p}", name=f"aoT{p}") for p in range(8)]

            with tc.tile_pool(name="xpool", bufs=1) as xpool, \
                 tc.tile_pool(name="wkp", bufs=1) as wkp, \
                 tc.tile_pool(name="wqp", bufs=1) as wqp:
                # ---- input DMAs, ordered by first use: x/wkv interleaved,
                # then rope-k, then rest of x, then cq/sq + wq, masks last.
                xT_sb = [None] * 8
                wkv_sb = [None] * 8

                def load_x(kb):
                    t = xpool.tile([128, SKV], BF16, tag=f"xT{kb}", name=f"xT{kb}")
                    eng = nc.sync if kb % 2 == 0 else nc.scalar
                    eng.dma_start(t[:], xT[kb * 128:(kb + 1) * 128, :])
                    xT_sb[kb] = t

                def load_wkv(kb):
                    t = wkp.tile([128, 512], BF16, tag=f"wkv{kb}", name=f"wkv{kb}")
                    eng = nc.scalar if kb % 2 == 0 else nc.sync
                    eng.dma_start(t[:], wkvT[kb * 128:(kb + 1) * 128, :])
                    wkv_sb[kb] = t

                for kb in range(4):
                    load_x(kb)
                    load_wkv(kb)
                cksk_sb = wkp.tile([128, 1024], F32, tag="cksk")
                nc.sync.dma_start(cksk_sb[:], ckskd)
                for kb in range(4, 8):
                    load_x(kb)
                    load_wkv(kb)
                cqsq_sb = wqp.tile([128, 512], F32, tag="cqsq")
                nc.sync.dma_start(cqsq_sb[:], cqsqd)
                wq_sb = []
                for kb in range(8):
                    t = wqp.tile([128, HID], BF16, tag=f"wq{kb}", name=f"wq{kb}")
                    nc.gpsimd.dma_start(t[:], wqT[kb * 128:(kb + 1) * 128, :])
                    wq_sb.append(t)

                # ======== phase A2: k/v projection + norm + rope + transpose ====
                with tc.tile_pool(name="tAk", bufs=3) as tA, \
                     tc.tile_pool(name="psmmk", bufs=4, space="PSUM") as psmm, \
                     tc.tile_pool(name="pstk", bufs=1, space="PSUM") as pst:

                    def k_rope(st, kvp):
                        # rope applied to the raw projection; the rms scale is
                        # per-(row,head) so it commutes to after the rotation
                        k1 = tA.tile([128, 256], F32, tag="k1", name=f"k1_{st}")
                        k1v = k1[:].rearrange("p (h d) -> p h d", h=KV)
                        kpv = kvp[:, 0:256].rearrange("p (h d) -> p h d", h=KV)
                        ck = cksk_sb[:, st * 64:(st + 1) * 64]
                        sk = cksk_sb[:, 512 + st * 64: 512 + (st + 1) * 64]
                        nc.vector.tensor_mul(
                            k1v, kpv, ck.unsqueeze(1).broadcast_to([128, KV, D]))
                        k2 = tA.tile([128, 256], F32, tag="k2", name=f"k2_{st}")
                        nc.vector.tensor_mul(
                            k2[:].rearrange("p (h a j) -> p h a j", h=KV, a=2),
                            kpv.rearrange("p h (a j) -> p h a j", a=2)[:, :, ::-1, :],
                            sk.rearrange("p (a j) -> p a j", a=2)
                            .unsqueeze(1).broadcast_to([128, KV, 2, 32]))
                        kr = tA.tile([128, 256], F32, tag="kr", name=f"kr{st}")
                        nc.vector.tensor_add(kr[:], k1[:], k2[:])
                        return kr

                    def k_stage2(st, kr, msk, kTps):
                        lnk = tA.tile([128, KV], F32, tag="lnk", name=f"lnk{st}")
                        nc.scalar.activation(lnk[:], msk[:], AF.Ln, bias=epsc[:],
                                             scale=1.0 / D)
                        invk = tA.tile([128, KV], F32, tag="invk", name=f"invk{st}")
                        nc.scalar.activation(invk[:], lnk[:], AF.Exp, scale=-0.5)
                        krs = tA.tile([128, 256], F32R, tag="krs", name=f"krs{st}")
                        nc.vector.tensor_mul(
                            krs[:].rearrange("p (h d) -> p h d", h=KV),
                            kr[:].rearrange("p (h d) -> p h d", h=KV),
                            invk[:].unsqueeze(2).broadcast_to([128, KV, D]))
                        j = st % 4
                        for kv in range(4):
                            sl = slice(kv * 64, (kv + 1) * 64)
                            nc.tensor.matmul(kTps[kv][:, j * 128:(j + 1) * 128],
                                             krs[:, sl], identr[:],
                                             is_transpose=True, start=True, stop=True)

                    def k_stage1(st):
                        kvp = psmm.tile([128, 512], F32, tag="kvp", name=f"kvp{st}")
                        for kb in range(8):
                            nc.tensor.matmul(kvp[:], xT_sb[kb][:, st * 128:(st + 1) * 128],
                                             wkv_sb[kb][:], start=(kb == 0), stop=(kb == 7))
                        nc.scalar.copy(
                            v_sb[st][:].rearrange("p (h d) -> p h d", d=65)[:, :, 0:64],
                            kvp[:, 256:512].rearrange("p (h d) -> p h d", d=64))
                        nc.gpsimd.dma_start(
                            v_sb[st][:].rearrange("p (h d) -> p h d", d=65)[:, :, 64:65],
                            vonesd[st * 128:(st + 1) * 128, 0:1].unsqueeze(1)
                            .broadcast_to([128, KV, 1]))
                        sqk = tA.tile([128, 256], F32, tag="sqk", name=f"sqk{st}")
                        nc.scalar.activation(sqk[:], kvp[:, 0:256], AF.Square)
                        msk = tA.tile([128, KV], F32, tag="msk", name=f"msk{st}")
                        nc.vector.tensor_reduce(
                            msk[:], sqk[:].rearrange("p (h d) -> p h d", h=KV),
                            axis=mybir.AxisListType.X, op=OP.add)
                        kr = k_rope(st, kvp)
                        return kr, msk

                    for sh in range(2):
                        kTps = [pst.tile([64, 512], F32R, tag=f"kTps{kv}",
                                         name=f"kTps{sh}_{kv}") for kv in range(4)]
                        pend = []
                        for st4 in range(4):
                            st = sh * 4 + st4
                            pend.append((st, *k_stage1(st)))
                            if len(pend) > 2:
                                k_stage2(*pend.pop(0), kTps)
                        for it in pend:
                            k_stage2(*it, kTps)
                        for kv in range(4):
                            nc.vector.tensor_copy(
                                kTd_sb[kv][0:64, sh * 512:(sh + 1) * 512], kTps[kv][:])
                            nc.gpsimd.dma_start(kTd_sb[kv][64:128, sh * 512:(sh + 1) * 512],
                                                kTd_sb[kv][0:64, sh * 512:(sh + 1) * 512])

                # ======== phase A1: q projection + norm + rope + transpose ======
                with tc.tile_pool(name="tAq", bufs=3) as tA, \
                     tc.tile_pool(name="psmmq", bufs=4, space="PSUM") as psmm, \
                     tc.tile_pool(name="pstq", bufs=1, space="PSUM") as pst:

                    def q_rope(st, hf, qp):
                        q1 = tA.tile([128, 512], F32, tag="q1", name=f"q1_{st}{hf}")
                        q1v = q1[:].rearrange("p (h d) -> p h d", h=8)
                        qpv = qp[:].rearrange("p (h d) -> p h d", h=8)
                        i = st - 4
                        ct = cqsq_sb[:, i * 64:(i + 1) * 64]
                        stt = cqsq_sb[:, 256 + i * 64: 256 + (i + 1) * 64]
                        nc.vector.tensor_mul(
                            q1v, qpv, ct.unsqueeze(1).broadcast_to([128, 8, D]))
                        q2 = tA.tile([128, 512], F32, tag="q2", name=f"q2_{st}{hf}")
                        nc.vector.tensor_mul(
                            q2[:].rearrange("p (h a j) -> p h a j", h=8, a=2),
                            qpv.rearrange("p h (a j) -> p h a j", a=2)[:, :, ::-1, :],
                            stt.rearrange("p (a j) -> p a j", a=2)
                            .unsqueeze(1).broadcast_to([128, 8, 2, 32]))
                        qr = tA.tile([128, 512], F32, tag="qr", name=f"qr{st}{hf}")
                        nc.vector.tensor_add(qr[:], q1[:], q2[:])
                        return qr

                    def q_stage2(st, hf, qr, msq, qTps):
                        lnq = tA.tile([128, 8], F32, tag="lnq", name=f"lnq{st}{hf}")
                        nc.scalar.activation(lnq[:], msq[:], AF.Ln, bias=epsc[:],
                                             scale=1.0 / D)
                        invq = tA.tile([128, 8], F32, tag="invq", name=f"invq{st}{hf}")
                        nc.scalar.activation(invq[:], lnq[:], AF.Exp, scale=-0.5)
                        qrs = tA.tile([128, 512], F32R, tag="qrs", name=f"qrs{st}{hf}")
                        nc.vector.tensor_mul(
                            qrs[:].rearrange("p (h d) -> p h d", h=8),
                            qr[:].rearrange("p (h d) -> p h d", h=8),
                            invq[:].unsqueeze(2).broadcast_to([128, 8, D]))
                        j = st - 4
                        for db4 in range(4):
                            sl = slice(db4 * 128, (db4 + 1) * 128)
                            nc.tensor.matmul(qTps[db4][:, j * 128:(j + 1) * 128],
                                             qrs[:, sl], identr[:],
                                             is_transpose=True, start=True, stop=True)

                    def q_stage1(st, hf):
                        qp = psmm.tile([128, 512], F32, tag="qp", name=f"qp{st}{hf}")
                        for kb in range(8):
                            nc.tensor.matmul(qp[:], xT_sb[kb][:, st * 128:(st + 1) * 128],
                                             wq_sb[kb][:, hf * 512:(hf + 1) * 512],
                                             start=(kb == 0), stop=(kb == 7))
                        sqq = tA.tile([128, 512], F32, tag="sqq", name=f"sqq{st}{hf}")
                        nc.scalar.activation(sqq[:], qp[:], AF.Square)
                        msq = tA.tile([128, 8], F32, tag="msq", name=f"msq{st}{hf}")
                        nc.vector.tensor_reduce(
                            msq[:], sqq[:].rearrange("p (h d) -> p h d", h=8),
                            axis=mybir.AxisListType.X, op=OP.add)
                        qr = q_rope(st, hf, qp)
                        return qr, msq

                    for hf in range(2):
                        qTps = [pst.tile([128, 512], F32R, tag=f"qTps{d}",
                                         name=f"qTps{hf}_{d}") for d in range(4)]
                        pend = []
                        for st in range(4, 8):
                            pend.append((st, hf, *q_stage1(st, hf)))
                            if len(pend) > 2:
                                q_stage2(*pend.pop(0), qTps)
                        for it in pend:
                            q_stage2(*it, qTps)
                        for db4 in range(4):
                            nc.vector.tensor_copy(qT_sb[hf * 4 + db4][:], qTps[db4][:])

            # ======== phase B: attention;  phase C: out-projection ========
            with tc.tile_pool(name="wB", bufs=1) as wB, \
                 tc.tile_pool(name="sbB", bufs=2) as sbB:
                masks = wB.tile([128, 4 * 256], BF16, tag="masks")
                nc.gpsimd.dma_start(masks[:], maskd)
                sinkr = wB.tile([1, 16 * 512], F32R, tag="sinkr")
                nc.gpsimd.dma_start(sinkr[:], sinkd)
                woT_sb = []
                for kb in range(8):
                    t = wB.tile([128, HID], BF16, tag=f"wo{kb}", name=f"wo{kb}")
                    nc.gpsimd.dma_start(t[:], woT[kb * 128:(kb + 1) * 128, :])
                    woT_sb.append(t)

                den_sb = wB.tile([16, 512], F32R, tag="den_sb")
                if "B" not in phases:
                    nc.vector.memset(den_sb[:].bitcast(F32), 1.0)
                    for p in range(8):
                        nc.vector.memset(aoT_sb[p][:].bitcast(F32), 0.0)
                rec_sb = wB.tile([16, 512], F32R, tag="rec_sb")
                with tc.tile_pool(name="psp", bufs=2, space="PSUM") as psp, \
                     tc.tile_pool(name="psav", bufs=2, space="PSUM") as psav:
                  for p in (range(8) if "B" in phases else []):
                      kv = p // 2
                      for Q in range(2):
                          psb = sbB.tile([128, 3072], BF16, tag="psb")
                          for h2 in range(2):
                              b = 64 * h2
                              ppA = psp.tile([128, 1024], F32, tag="ppA")
                              ppB = psp.tile([128, 512], F32, tag="ppB")
                              for nu in range(6):
                                  kap = 2 * Q + nu
                                  lhsT = kTd_sb[kv][b:b + 64, kap * 128:(kap + 1) * 128]
                                  rhs = qT_sb[p][b:b + 64, Q * 256:(Q + 1) * 256]
                                  tgt = (ppA[:, nu * 256:(nu + 1) * 256] if nu < 4
                                         else ppB[:, (nu - 4) * 256:(nu - 3) * 256])
                                  nc.tensor.matmul(tgt, lhsT, rhs,
                                                   start=True, stop=True)
                              nc.scalar.activation(psb[:, h2 * 1536: h2 * 1536 + 1024],
                                                   ppA[:], AF.Exp, scale=SCALE)
                              nc.scalar.activation(psb[:, h2 * 1536 + 1024: h2 * 1536 + 1536],
                                                   ppB[:], AF.Exp, scale=SCALE)
                              # zero the disallowed positions (binary mask) on DVE
                              nc.vector.tensor_mul(
                                  psb[:, h2 * 1536: h2 * 1536 + 512],
                                  psb[:, h2 * 1536: h2 * 1536 + 512],
                                  masks[:, 0:512])
                              nc.vector.tensor_mul(
                                  psb[:, h2 * 1536 + 1024: h2 * 1536 + 1536],
                                  psb[:, h2 * 1536 + 1024: h2 * 1536 + 1536],
                                  masks[:, 512:1024])
                          avp = psav.tile([65, 512], F32, tag="avp")
                          for h2 in range(2):
                              for nu in range(6):
                                  stk = 2 * Q + nu
                                  rhs = psb[:, h2 * 1536 + nu * 256: h2 * 1536 + (nu + 1) * 256]
                                  nc.tensor.matmul(avp[:, h2 * 256:(h2 + 1) * 256],
                                                   v_sb[stk][:, kv * 65:(kv + 1) * 65], rhs,
                                                   start=(nu == 0), stop=False)
                              # sink + core-0 halo correction folded into the
                              # denominator row (row 64) via a rank-1 matmul
                              nc.tensor.matmul(avp[:, h2 * 256:(h2 + 1) * 256],
                                               delta65[:],
                                               sinkr[0:1, (2 * p + Q) * 512 + h2 * 256:
                                                     (2 * p + Q) * 512 + (h2 + 1) * 256],
                                               start=False, stop=True)
                          # drain psum, collect denominators, and move av halves
                          # into the transposed head layout (partition relocation
                          # must go through an SBUF-source DMA)
                          av2 = sbB.tile([64, 512], BF16, tag="av2")
                          nc.vector.tensor_copy(av2[:], avp[0:64, :])
                          denrow = sbB.tile([65, 512], F32R, tag="denrow")
                          nc.scalar.copy(denrow[64:65, :].bitcast(F32), avp[64:65, :])
                          nc.sync.dma_start(den_sb[2 * p + Q: 2 * p + Q + 1, :],
                                            denrow[64:65, :])
                          nc.sync.dma_start(aoT_sb[p][0:64, Q * 256:(Q + 1) * 256],
                                            av2[:, 0:256])
                          nc.sync.dma_start(aoT_sb[p][64:128, Q * 256:(Q + 1) * 256],
                                            av2[:, 256:512])
                # batched reciprocal of all 32 denominators, then normalize in place
                nc.vector.reciprocal(rec_sb[:].bitcast(F32), den_sb[:].bitcast(F32))
                rec1 = wB.tile([1, 16 * 512], F32R, tag="rec1")
                nc.sync.dma_start(rec1[:], rec_sb[:])
                onesrow = wB.tile([1, 128], F32R, tag="onesrow")
                nc.vector.memset(onesrow[:].bitcast(F32), 1.0)
                with tc.tile_pool(name="psrep", bufs=2, space="PSUM") as psrep:
                  for p in range(8):
                    for Q in range(2):
                        base = (2 * p + Q) * 512
                        repA = psrep.tile([128, 256], F32, tag="repA")
                        repB = psrep.tile([128, 256], F32, tag="repB")
                        nc.tensor.matmul(repA[:], onesrow[:],
                                         rec1[0:1, base: base + 256])
                        nc.tensor.matmul(repB[:], onesrow[:],
                                         rec1[0:1, base + 256: base + 512])
                        nc.vector.tensor_mul(aoT_sb[p][0:64, Q * 256:(Q + 1) * 256],
                                             aoT_sb[p][0:64, Q * 256:(Q + 1) * 256],
                                             repA[0:64, :])
                        nc.vector.tensor_mul(aoT_sb[p][64:128, Q * 256:(Q + 1) * 256],
                                             aoT_sb[p][64:128, Q * 256:(Q + 1) * 256],
                                             repB[64:128, :])

                with tc.tile_pool(name="psC", bufs=2, space="PSUM") as psC:
                  for sblk in (range(4) if "C" in phases else []):
                    for nh in range(2):
                        op = psC.tile([128, 512], F32, tag="op")
                        for kb in range(8):
                            nc.tensor.matmul(op[:], aoT_sb[kb][:, sblk * 128:(sblk + 1) * 128],
                                             woT_sb[kb][:, nh * 512:(nh + 1) * 512],
                                             start=(kb == 0), stop=(kb == 7))
                        osb = sbB.tile([128, 512], F32, tag="osb")
                        nc.scalar.copy(osb[:], op[:])
                        eng = nc.sync if (sblk + nh) % 2 == 0 else nc.scalar
                        eng.dma_start(
                            outd[sblk * 128:(sblk + 1) * 128, nh * 512:(nh + 1) * 512],
                            osb[:])

    nc.compile()
    return nc


def _prep_inputs(x, cos, sin, wq, wk, wv, wo, q_norm_w, k_norm_w, sinks):
    """Build the 8 per-core input maps."""
    x = np.asarray(x, np.float32).reshape(S, HID)
    cos = np.asarray(cos, np.float32)
    sin = np.asarray(sin, np.float32)
    wq = np.asarray(wq, np.float32)
    wk = np.asarray(wk, np.float32)
    wv = np.asarray(wv, np.float32)
    wo = np.asarray(wo, np.float32)
    qw = np.asarray(q_norm_w, np.float32)
    kw = np.asarray(k_norm_w, np.float32)
    sinks = np.asarray(sinks, np.float32)

    wqT = np.ascontiguousarray(wq.T)                      # [HID, H*D]
    wkvT = np.ascontiguousarray(np.concatenate([wk, wv], 0).T)  # [HID, 512]
    woT = np.ascontiguousarray(wo.T)                      # [H*D, HID]
    ident = np.eye(128, dtype=np.float32)

    # rope coefficient tables with norm weight folded in
    # q'' [d] = qn[d]*w[d]*cos[d] + rot(qn*w)[d]*sin[d]
    #   rot(qn*w)[d<32] = -qn[d+32]*w[d+32]; rot[d>=32] = qn[d-32]*w[d-32]
    sgn = np.concatenate([-np.ones(32, np.float32), np.ones(32, np.float32)])
    wrot_q = np.concatenate([qw[32:], qw[:32]])
    wrot_k = np.concatenate([kw[32:], kw[:32]])
    cw_q = cos * qw[None, :]
    sw_q = sin * (sgn * wrot_q)[None, :]
    cw_k = cos * kw[None, :]
    sw_k = sin * (sgn * wrot_k)[None, :]

    # additive masks for partial nu blocks (order nu=0,1,4,5)
    r = np.arange(128)[:, None]
    c = np.arange(256)[None, :]
    mstack = []
    for nu in (0, 1, 4, 5):
        ij = c - r + 512 - 128 * nu
        allowed = (ij >= 0) & (ij < WINDOW)
        mstack.append(np.where(allowed, 1.0, 0.0).astype(np.float32))
    masks = np.concatenate(mstack, 1)                     # [128, 1024]

    xT = np.ascontiguousarray(x.T)                        # [HID, S]
    esink = np.exp(sinks.astype(np.float64)).astype(np.float32)

    in_maps = []
    for core in range(NCORE):
        start = SLOC * core
        lo = start - WINDOW
        xt_loc = np.zeros((HID, SKV), np.float32)
        srclo = max(0, lo)
        xt_loc[:, srclo - lo:] = xT[:, srclo:start + SLOC]
        idx_k = np.clip(np.arange(lo, start + SLOC), 0, S - 1)
        # sink rhs row: col (2p+Q)*512 + h2*256 + qq -> exp(sink[2p+h2]).
        # core 0: halo keys' denominator contributions are suppressed by
        # zeroing their v ones-column (vones), as large partial sums would
        # hit the reduced-precision psum accumulate.
        sink_rhs = np.zeros((1, 8192), np.float32)
        for p in range(8):
            for Qb in range(2):
                for h2 in range(2):
                    base = (2 * p + Qb) * 512 + h2 * 256
                    sink_rhs[0, base:base + 256] = esink[2 * p + h2]
        vones = np.ones((SKV, 1), np.float32)
        if core == 0:
            vones[:WINDOW] = 0.0
        # packed rope tables: [128, st-blocks*64] (cos blocks then sin blocks)
        cq_loc = cw_q[start:start + SLOC].reshape(4, 128, 64)
        sq_loc = sw_q[start:start + SLOC].reshape(4, 128, 64)
        cqsq = np.concatenate(
            [cq_loc[i] for i in range(4)] + [sq_loc[i] for i in range(4)], axis=1)
        ck_loc = cw_k[idx_k].reshape(8, 128, 64)
        sk_loc = sw_k[idx_k].reshape(8, 128, 64)
        cksk = np.concatenate(
            [ck_loc[i] for i in range(8)] + [sk_loc[i] for i in range(8)], axis=1)
        in_maps.append(dict(
            xT=xt_loc.astype(BF16NP),
            wqT=wqT.astype(BF16NP), wkvT=wkvT.astype(BF16NP),
            woT=woT.astype(BF16NP),
            cqsq=np.ascontiguousarray(cqsq),
            cksk=np.ascontiguousarray(cksk),
            masks=masks.astype(BF16NP), ident=ident.astype(BF16NP),
            identr=ident,
            sink_rhs=sink_rhs, vones=vones.astype(BF16NP),
        ))
    return in_maps


def kernel(x, cos, sin, wq, wk, wv, wo, q_norm_w, k_norm_w, sinks, **kw):
    if "nc" not in _cache:
        _cache["nc"] = _build()
    nc = _cache["nc"]
    in_maps = _prep_inputs(x, cos, sin, wq, wk, wv, wo, q_norm_w, k_norm_w, sinks)
    res = run_bass_kernel_spmd(nc, in_maps, core_ids=list(range(NCORE)), **kw)
    out = np.empty((S, HID), np.float32)
    for core in range(NCORE):
        out[core * SLOC:(core + 1) * SLOC] = res.results[core]["out"]
    if kw:
        _cache["last_results"] = res
    return out.reshape(B, S, HID)
